# revision 1
# baseline (speedup 1.0000x reference)
"""Single-head causal attention (B=4, S=4096, d_e=512, d_k=d_v=64) on 8 TRN2 cores.

SPMD: one program on all 8 cores; per-core behavior driven purely by input data.
  - core c handles batch b=c//2; the two cores of a batch split the 8 q-tiles
    (512 queries each) load-balanced: parity 0 -> q-tiles {0,2,5,7}, parity 1 ->
    {1,3,4,6} (equal causal work: 18 kv-tile interactions each, padded to 20).
  - Inputs are host-tiled so each s-tile / q-tile arrives in ONE DMA (DMA issue
    costs ~1.2us of sequencer time each); issue is spread across the SP/ACT
    queues and ordered so each group's tiles land just-in-time.
  - Attention in "scores^T" layout: st[s,q] = k @ (q/sqrt(dk))^T. The softmax
    denominator rides the AV matmul via an appended ones column on v. Causal /
    padding masks are multiplicative {0,1} tiles picked from an SBUF palette at
    data-driven dynamic offsets, applied post-exp on DVE/GPSIMD with the
    masked pairs' AV matmuls deferred to the group tail so mask latency
    stays off the PE chain.
  - All large matmuls in float32r (single-pass PE, ~1e-4 rel err); score pairs
    are row-stacked on the PE array via tile_position (K=64 each) so two score
    matmuls run concurrently.
  - Per-group PSUM accumulation releases early (one copy to SBUF); the heavy
    finalize (PE-transpose out^T [65,512] -> [512,65], multiply by reciprocal
    of the denominator column, one DMA per group) runs as a tail phase.
    Host scatters the per-core q-tiles into [B,S,64].

Measured on 8 axon-attached TRN2 cores: ~83-91 us steady-state per
invocation (For_i loop delta method with staggered_reset so consecutive
invocations pipeline; cost-model timeline predicts 84 us for a single
shot), output rel err ~4.4e-4 vs the fp32 reference (float32r matmul
precision). Next-group k/v/q projections are prefetched mid-group so
group starts never stall the PE queue on DMA.
"""
import numpy as np
from contextlib import ExitStack

import concourse.bass as bass
import concourse.tile as tile
from concourse import bacc, mybir
from concourse.bass_utils import run_bass_kernel_spmd

f32 = mybir.dt.float32
f32r = mybir.dt.float32r
i32 = mybir.dt.int32
AF = mybir.ActivationFunctionType
ET = mybir.EngineType

B, S, DE, DK, DV = 4, 4096, 512, 64, 64
QT = 512                 # queries per group
NT = S // QT             # 8 s/q tiles per batch
NG = 4                   # groups (q-tiles) per core
NCH = DE // 128          # 4 contraction chunks
TW = NCH * QT            # tile width in sbuf cols (2048)
TQ = [[0, 2, 5, 7], [1, 3, 4, 6]]   # parity -> group -> q_tile index

# palette column offsets (elements): [drop(512) | keep(512) | tri master(896)]
PAL_DROP = 0
PAL_KEEP = 512
PAL_TRI0 = 1024 + 384    # tri for block blk is PAL_TRI0 - 128*blk


def build(kiter: int = 1):
    nc = bacc.Bacc("TRN2", target_bir_lowering=False, debug=False)

    xt_d = nc.dram_tensor("xt", [NT, 128, TW], f32r, kind="ExternalInput").ap()
    xq_d = nc.dram_tensor("xq", [NG, 128, TW], f32r, kind="ExternalInput").ap()
    wkv_d = nc.dram_tensor("wkv", [128, NCH * 128], f32r, kind="ExternalInput").ap()
    wq_d = nc.dram_tensor("wq", [128, NCH * DK], f32r, kind="ExternalInput").ap()
    moff_d = nc.dram_tensor("moff", [1, 32], i32, kind="ExternalInput").ap()
    tri_d = nc.dram_tensor("tri", [128, 896], f32r, kind="ExternalInput").ap()
    ident_d = nc.dram_tensor("ident", [128, 128], f32, kind="ExternalInput").ap()
    out_d = nc.dram_tensor("out", [NG, 128, 4 * DV], f32, kind="ExternalOutput").ap()

    with tile.TileContext(nc) as tc, ExitStack() as ctx:

        def body():
            per = ctx.enter_context(tc.tile_pool(name="persist", bufs=1))
            # PSUM pools: pkvt 2x[128,512] + pqo 2x[65,512] + ps 2x[128,1024]
            # = 8 banks, all coexisting (no cross-phase overlap deps)
            pkv_pool = ctx.enter_context(tc.tile_pool(name="pkv", bufs=2, space="PSUM"))
            pq_pool = ctx.enter_context(tc.tile_pool(name="pq", bufs=2, space="PSUM"))
            ps_pool = ctx.enter_context(tc.tile_pool(name="ps", bufs=2, space="PSUM"))

            vts_pool = ctx.enter_context(tc.tile_pool(name="vts", bufs=2))
            exp_pool = ctx.enter_context(tc.tile_pool(name="exp", bufs=8))
            fin_pool = ctx.enter_context(tc.tile_pool(name="fin", bufs=2))

            xts = per.tile([128, NT * TW], f32r)           # x^T, tile-major
            xqs = per.tile([128, NG * TW], f32r)           # x^T own q-tiles
            wkv = per.tile([128, NCH * 128], f32r)
            wq = per.tile([128, NCH * DK], f32r)
            ident = per.tile([128, 128], f32)
            pal = per.tile([128, 1920], f32r)
            kT = per.tile([128, S], f32r)   # rows 0:64 and 64:128 both hold k^T
            vaug = per.tile([128, (S // 128) * 65], f32r)  # 32 x [128,65]
            qTg = per.tile([128, NG * QT], f32r)  # duplicated rows like kT
            oTall = per.tile([65, NG * QT], f32)
            mofft = per.tile([1, 32], i32)

            # DMA emission order == transfer priority: interleave xq/xt so
            # each group's tiles land just before they are needed. xq/consts
            # ride the ACT queue, xt the SP queue.
            nc.sync.dma_start(xqs[:, bass.ts(0, TW)], xq_d[0])
            nc.scalar.dma_start(wkv[:], wkv_d[:])
            nc.scalar.dma_start(wq[:], wq_d[:])
            nc.scalar.dma_start(mofft[:], moff_d[:])
            nc.sync.dma_start(xts[:, bass.ts(0, TW)], xt_d[0])
            nc.sync.dma_start(xts[:, bass.ts(1, TW)], xt_d[1])
            nc.scalar.dma_start(xqs[:, bass.ts(1, TW)], xq_d[1])
            nc.scalar.dma_start(ident[:], ident_d[:])
            nc.scalar.dma_start(pal[:, 1024:1920], tri_d[:])
            nc.sync.dma_start(xts[:, bass.ts(2, TW)], xt_d[2])
            nc.sync.dma_start(xts[:, bass.ts(3, TW)], xt_d[3])
            nc.scalar.dma_start(xqs[:, bass.ts(2, TW)], xq_d[2])
            nc.sync.dma_start(xts[:, bass.ts(4, TW)], xt_d[4])
            nc.sync.dma_start(xts[:, bass.ts(5, TW)], xt_d[5])
            nc.scalar.dma_start(xqs[:, bass.ts(3, TW)], xq_d[3])
            nc.sync.dma_start(xts[:, bass.ts(6, TW)], xt_d[6])
            nc.sync.dma_start(xts[:, bass.ts(7, TW)], xt_d[7])
            nc.gpsimd.memset(pal[:, 0:512].bitcast(f32), 0.0)
            nc.gpsimd.memset(pal[:, 512:1024].bitcast(f32), 1.0)

            # ---- projections ------------------------------------------------
            def q_proj(g):
                pq_t = pq_pool.tile([65, QT], f32, tag="pqo")
                pq = pq_t[0:64, :]
                for c in range(NCH):
                    nc.tensor.matmul(pq[:], wq[:, bass.ts(c, DK)],
                                     xqs[:, g * TW + c * QT: g * TW + (c + 1) * QT],
                                     start=(c == 0), stop=(c == NCH - 1))
                nc.vector.tensor_copy(qTg[0:64, bass.ts(g, QT)], pq[:])
                nc.vector.tensor_copy(qTg[64:128, bass.ts(g, QT)], pq[:])

            # k^T and v_aug for one s-tile
            def kv_proj(t):
                pkv = pkv_pool.tile([128, QT], f32, tag="pkvt")
                for c in range(NCH):
                    nc.tensor.matmul(pkv[:], wkv[:, bass.ts(c, 128)],
                                     xts[:, t * TW + c * QT: t * TW + (c + 1) * QT],
                                     start=(c == 0), stop=(c == NCH - 1))
                nc.vector.tensor_copy(kT[0:64, bass.ts(t, QT)], pkv[0:64, :])
                nc.vector.tensor_copy(kT[64:128, bass.ts(t, QT)], pkv[0:64, :])
                vts = vts_pool.tile([65, QT], f32, tag="vts")
                nc.vector.tensor_copy(vts[0:64, :], pkv[64:128, :])
                nc.vector.memset(vts[64:65, :], 1.0)
                pvt = pkv_pool.tile([128, 4 * 65], f32, tag="pkvt")
                for blk in range(4):
                    nc.tensor.transpose(pvt[:, bass.ts(blk, 65)],
                                        vts[:, bass.ts(blk, 128)],
                                        ident[0:65, 0:65])
                nc.vector.tensor_copy(vaug[:, t * 4 * 65:(t + 1) * 4 * 65], pvt[:])

            q_proj(0)

            # mask palette offsets: rel even -> used by DVE, odd -> by POOL
            mv = [nc.values_load(mofft[0:1, i:i + 1].to_broadcast((1, 1)),
                                 engines=[ET.DVE if (i % 2 == 0) else ET.Pool],
                                 min_val=0, max_val=1920 - 512,
                                 skip_runtime_bounds_check=True)
                  for i in range(32)]

            # ---- attention, group-major, kv-projections just-in-time --------
            def finalize(g):
                pt = pkv_pool.tile([128, 4 * 65], f32, tag="pkvt")
                for blk in range(4):
                    nc.tensor.transpose(pt[:, bass.ts(blk, 65)],
                                        oTall[:, g * QT + blk * 128:
                                              g * QT + (blk + 1) * 128],
                                        ident[0:65, 0:65])
                onat = fin_pool.tile([128, 4 * 65], f32)
                nc.vector.tensor_copy(onat[:], pt[:])
                ofin = fin_pool.tile([128, 4 * DV], f32)
                for blk in range(4):
                    rec = fin_pool.tile([128, 1], f32)
                    nc.vector.reciprocal(rec[:], onat[:, blk * 65 + 64: blk * 65 + 65])
                    nc.vector.tensor_scalar_mul(
                        ofin[:, bass.ts(blk, DV)], onat[:, blk * 65: blk * 65 + 64],
                        rec[:])
                nc.sync.dma_start(out_d[g], ofin[:])

            q_proj(1)
            kv_proj(0)
            kv_proj(1)
            for g in range(NG):
                npairs = 4 * g + 4
                po = pq_pool.tile([65, QT], f32, tag="pqo")
                # masked pairs (last 4 in index space) run first; their AV
                # matmuls are deferred to the group tail so the mask-multiply
                # latency stays off the PE chain.
                unm = list(range(4 * g))
                msk = list(range(4 * g, 4 * g + 4))
                order = (unm[:-1] + msk + unm[-1:]) if unm else msk
                av_emitted = [0]
                n_av = 2 * npairs
                deferred = []

                def emit_av(pi, em, g=g, po=po, av_emitted=av_emitted, n_av=n_av):
                    for half in range(2):
                        sb = 2 * pi + half
                        nc.tensor.matmul(po[:], vaug[:, sb * 65:(sb + 1) * 65],
                                         em[:, bass.ts(half, QT)],
                                         start=(av_emitted[0] == 0),
                                         stop=(av_emitted[0] == n_av - 2))
                        av_emitted[0] += 2

                def flush(pend, g=g, emit_av=emit_av, deferred=deferred):
                    ps, pi = pend
                    em = exp_pool.tile([128, 2 * QT], f32r)
                    nc.scalar.activation(em[:], ps[:], AF.Exp)
                    if pi >= 4 * g:   # masked pair: mults split DVE/POOL
                        for half in range(2):
                            rel = 2 * pi + half - 8 * g
                            eng = nc.vector if half == 0 else nc.gpsimd
                            eng.tensor_mul(
                                em[:, bass.ts(half, QT)],
                                em[:, bass.ts(half, QT)],
                                pal[:, bass.ds(mv[g * 8 + rel], QT)])
                        deferred.append((pi, em))
                    else:
                        emit_av(pi, em)

                pending = None
                for pos, pi in enumerate(order):
                    ps = ps_pool.tile([128, 2 * QT], f32)
                    for half in range(2):
                        sb = 2 * pi + half
                        rows = slice(64 * half, 64 * half + 64)
                        nc.tensor.matmul(ps[:, bass.ts(half, QT)],
                                         kT[rows, bass.ts(sb, 128)],
                                         qTg[rows, bass.ts(g, QT)],
                                         start=True, stop=True,
                                         tile_position=(64 * half, 0))
                    if pending is not None:
                        flush(pending)
                    pending = (ps, pi)
                    # prefetch next group's projections behind the first pairs
                    if g < NG - 1:
                        if pos == 0:
                            kv_proj(2 * g + 2)
                        elif pos == 1:
                            kv_proj(2 * g + 3)
                        elif pos == 2:
                            q_proj(g + 1) if g + 1 >= 2 else None
                flush(pending)
                for pi, em in deferred:
                    emit_av(pi, em)

                # release po immediately; heavy finalize deferred to the tail
                nc.vector.tensor_copy(oTall[:, bass.ts(g, QT)], po[:])

            for g in range(NG):
                finalize(g)

        if kiter == 1:
            body()
        else:
            with tc.For_i(0, kiter, 1, staggered_reset=True):
                body()

    nc.compile()
    return nc


def _tile_cols(a):
    """[512, n*512] (d_e, cols) -> [n, 128, 4*512] tile-major host layout."""
    de, w = a.shape
    n = w // QT
    # out[t, p, c*QT + s] = a[c*128 + p, t*QT + s]
    return np.ascontiguousarray(
        a.reshape(NCH, 128, n, QT).transpose(2, 1, 0, 3).reshape(n, 128, NCH * QT))


def make_inputs(x, Wq, Wk, Wv):
    """Per-core input maps. x:[B,S,DE] f32; W*: [DE,64] f32."""
    wkv = np.concatenate([Wk, Wv], axis=1).astype(np.float32)          # [512,128]
    wqs = (Wq / np.float32(np.sqrt(DK))).astype(np.float32)            # [512,64]
    # weights chunk-major: [128, c*width + j] = W[c*128 + p, j]
    wkv_h = np.ascontiguousarray(
        wkv.reshape(NCH, 128, 128).transpose(1, 0, 2).reshape(128, NCH * 128))
    wq_h = np.ascontiguousarray(
        wqs.reshape(NCH, 128, DK).transpose(1, 0, 2).reshape(128, NCH * DK))
    ident = np.eye(128, dtype=np.float32)
    tri = (np.arange(896)[None, :] >= np.arange(128)[:, None] + 384).astype(np.float32)
    in_maps = []
    for core in range(8):
        b, p = core // 2, core % 2
        xt = np.ascontiguousarray(x[b].T, dtype=np.float32)            # [512, 4096]
        cols = np.concatenate([np.arange(t * QT, (t + 1) * QT) for t in TQ[p]])
        moff = np.zeros((1, 32), dtype=np.int32)
        for g in range(NG):
            t = TQ[p][g]
            for rel in range(8):
                j = 2 * g + rel // 4
                blk = rel % 4
                if j < t:
                    moff[0, g * 8 + rel] = PAL_KEEP
                elif j == t:
                    moff[0, g * 8 + rel] = PAL_TRI0 - 128 * blk
                else:
                    moff[0, g * 8 + rel] = PAL_DROP
        in_maps.append(dict(xt=_tile_cols(xt), xq=_tile_cols(xt[:, cols]),
                            wkv=wkv_h, wq=wq_h, moff=moff, tri=tri, ident=ident))
    return in_maps


def assemble(results):
    out = np.empty((B, S, DV), dtype=np.float32)
    for core in range(8):
        b, p = core // 2, core % 2
        o = results[core]["out"]                      # [NG, 128, 4*64]
        for g in range(NG):
            t = TQ[p][g]
            # query q = blk*128 + p_row lives at o[g][p_row, blk*64:(blk+1)*64]
            blk_view = o[g].reshape(128, 4, DV).transpose(1, 0, 2)   # [blk,p,dv]
            out[b, t * QT:(t + 1) * QT, :] = blk_view.reshape(QT, DV)
    return out


_cache = {}


def _get_nc(kiter=1):
    if kiter not in _cache:
        _cache[kiter] = build(kiter)
    return _cache[kiter]


def run(x, Wq, Wk, Wv, kiter=1):
    nc = _get_nc(kiter)
    in_maps = make_inputs(x, Wq, Wk, Wv)
    res = run_bass_kernel_spmd(nc, in_maps, list(range(8)))
    return assemble(res.results)


def kernel(x, Wq, Wk, Wv):
    x = np.asarray(x, dtype=np.float32)
    return run(x, np.asarray(Wq, np.float32), np.asarray(Wk, np.float32),
               np.asarray(Wv, np.float32))



# revision 36
# speedup vs baseline: 7.3536x; 7.3536x over previous
"""Single-head causal attention (B=4, S=4096, d_e=512, d_k=d_v=64) on 8 TRN2 cores.

SPMD: one program on all 8 cores; per-core behavior driven purely by input data.
  - core c handles batch b=c//2; the two cores of a batch split the 8 q-tiles
    (512 queries each) load-balanced: parity 0 -> q-tiles {0,2,5,7}, parity 1 ->
    {1,3,4,6} (equal causal work: 18 kv-tile interactions each, padded to 20 --
    provably minimal for any SPMD-uniform per-position pair budget).
  - All PE/DVE data is bf16 (halves HBM + on-chip traffic; PSUM stays f32).
    The steady state is ACT-exp-throughput-bound (~1.04us per 256kv x 512q
    score pair), so the whole kernel is built around keeping the exp stream
    dense: a single flat 40-pair pipeline across group boundaries, projections
    (kv 4-matmul chunks, v-transposes, q) back-loaded into late exp-heavy
    groups, and the kv projection split into matmul+copy vs transpose phases
    so the vts round-trip hides behind score pairs.
  - Attention in "scores^T" layout: st[s,q] = k @ (q/sqrt(dk))^T. The softmax
    denominator rides the AV matmul via an appended ones column on v (vaug
    blocks are 66-wide so bf16 PSUM transposes stay 4-byte aligned). Causal /
    padding masks are multiplicative {0,1} bf16 tiles picked from an SBUF
    palette at data-driven dynamic offsets (2 base register offsets per group,
    block shifts are static -128*b arithmetic), applied post-exp on DVE with
    each masked pair's AV matmuls deferred exactly one flush slot.
  - DMA issues cost ~1.5us of sequencer time each, so they are kept off the
    ACT queue past the first 4 (ACT dispatches exp) and spread SP/ACT in
    landing-priority order; moff/ident ride Pool's software DGE.
  - Output leaves the device unnormalized as bf16 [65, 512] per group (64 AV
    rows + denominator row); the host divides and transposes during assembly.

Single-shot cost-model timeline: 63.8us (baseline f32r kernel: 84.5us).
Hardware-verified rel err 6.4e-3 vs the fp32 reference (bf16 data path).
GOTCHAS (hardware-verified): GPSIMD cannot touch PSUM; PSUM accesses must be
4B-aligned (bf16 odd-element offsets are not); gpsimd memset on bf16 writes
32-bit patterns (use DVE); engine partition starts must be multiples of 32.
"""
import numpy as np
import ml_dtypes
from contextlib import ExitStack

import concourse.bass as bass
import concourse.tile as tile
from concourse import bacc, mybir
from concourse.bass_utils import run_bass_kernel_spmd

f32 = mybir.dt.float32
bf16 = mybir.dt.bfloat16
i32 = mybir.dt.int32
AF = mybir.ActivationFunctionType
ET = mybir.EngineType
bfloat16 = ml_dtypes.bfloat16

B, S, DE, DK, DV = 4, 4096, 512, 64, 64
QT = 512                 # queries per group
NT = S // QT             # 8 s/q tiles per batch
NG = 4                   # groups (q-tiles) per core
NCH = DE // 128          # 4 contraction chunks
TW = NCH * QT            # tile width in sbuf cols (2048)
TQ = [[0, 2, 5, 7], [1, 3, 4, 6]]   # parity -> group -> q_tile index

# palette bases (element cols): [drop(896) | keep(896) | tri(896)]
# window for block b is base - 128*b, so bases sit +384 into each region.
PAL_DROP = 384
PAL_KEEP = 896 + 384
PAL_TRI = 2 * 896 + 384
PAL_W = 3 * 896


def build(kiter: int = 1):
    nc = bacc.Bacc("TRN2", target_bir_lowering=False, debug=False)

    xt_d = nc.dram_tensor("xt", [NT, 128, TW], bf16, kind="ExternalInput").ap()
    xq_d = nc.dram_tensor("xq", [NG, 128, TW], bf16, kind="ExternalInput").ap()
    wb_d = nc.dram_tensor("wb", [128, NCH * (128 + DK)], bf16,
                          kind="ExternalInput").ap()
    moff_d = nc.dram_tensor("moff", [1, 8], i32, kind="ExternalInput").ap()
    tri_d = nc.dram_tensor("tri", [128, 896], bf16, kind="ExternalInput").ap()
    ident_d = nc.dram_tensor("ident", [66, 66], bf16, kind="ExternalInput").ap()
    out_d = nc.dram_tensor("out", [NG, 65, QT], bf16, kind="ExternalOutput").ap()

    with tile.TileContext(nc) as tc, ExitStack() as ctx:

        def body():
            per = ctx.enter_context(tc.tile_pool(name="persist", bufs=1))
            pkv_pool = ctx.enter_context(tc.tile_pool(name="pkv", bufs=2, space="PSUM"))
            pq_pool = ctx.enter_context(tc.tile_pool(name="pq", bufs=2, space="PSUM"))
            ps_pool = ctx.enter_context(tc.tile_pool(name="ps", bufs=2, space="PSUM"))

            exp_pool = ctx.enter_context(tc.tile_pool(name="exp", bufs=8))
            fin_pool = ctx.enter_context(tc.tile_pool(name="fin", bufs=2))

            xts = per.tile([128, NT * TW], bf16)           # x^T, tile-major
            xqs = per.tile([128, NG * TW], bf16)           # x^T own q-tiles
            wb = per.tile([128, NCH * (128 + DK)], bf16)   # [wkv(4x128)|wq(4x64)]
            ident = per.tile([66, 66], bf16)
            pal = per.tile([128, PAL_W], bf16)
            kT = per.tile([128, S], bf16)   # rows 0:64 and 64:128 both hold k^T
            vaug = per.tile([128, (S // 128) * 66], bf16)  # 32 x [128,66(65 used)]
            vts = per.tile([66, 2 * QT], bf16)             # ping-pong v rows
            qTg = per.tile([128, NG * QT], bf16)  # duplicated rows like kT
            mofft = per.tile([1, 8], i32)

            # DMA emission order == transfer priority. moff rides the cheap
            # Pool queue; xq/xt round-robin between SP and ACT queues so the
            # HWDGE interleaves them in just-in-time order.
            # DMA issues cost ~1.5us of sequencer time each. SP and ACT
            # alternate so HWDGE interleaves transfers in need order, but ACT
            # gets only the first 4 issues so its queue is clear well before
            # the first exp dispatch (~10.5us); SP absorbs the rest.
            nc.sync.dma_start(wb[:], wb_d[:])
            nc.scalar.dma_start(xqs[:, bass.ts(0, TW)], xq_d[0])
            nc.sync.dma_start(xts[:, bass.ts(0, TW)], xt_d[0])
            nc.scalar.dma_start(xts[:, bass.ts(1, TW)], xt_d[1])
            nc.sync.dma_start(xqs[:, bass.ts(1, TW)], xq_d[1])
            nc.scalar.dma_start(xts[:, bass.ts(2, TW)], xt_d[2])
            nc.sync.dma_start(pal[:, 2 * 896:3 * 896], tri_d[:])
            nc.scalar.dma_start(xts[:, bass.ts(3, TW)], xt_d[3])
            nc.sync.dma_start(xts[:, bass.ts(4, TW)], xt_d[4])
            nc.sync.dma_start(xqs[:, bass.ts(2, TW)], xq_d[2])
            nc.sync.dma_start(xts[:, bass.ts(5, TW)], xt_d[5])
            nc.sync.dma_start(xts[:, bass.ts(6, TW)], xt_d[6])
            nc.sync.dma_start(xts[:, bass.ts(7, TW)], xt_d[7])
            nc.sync.dma_start(xqs[:, bass.ts(3, TW)], xq_d[3])
            nc.gpsimd.dma_start(mofft[:], moff_d[:])
            nc.gpsimd.dma_start(ident[:], ident_d[:])
            nc.vector.memset(pal[:, 0:896], 0.0)
            nc.vector.memset(pal[:, 896:2 * 896], 1.0)
            nc.vector.memset(vts[64:66, :], 0.0)
            nc.vector.memset(vts[64:65, :], 1.0)

            # mask palette base offsets, loaded JIT in pairs (2 per group)
            mv = [None] * 8

            def load_mv(g):
                for j in range(2):
                    mv[2 * g + j] = nc.values_load(
                        mofft[0:1, 2 * g + j:2 * g + j + 1].to_broadcast((1, 1)),
                        engines=[ET.DVE], min_val=PAL_DROP, max_val=PAL_TRI,
                        skip_runtime_bounds_check=True)

            # ---- projections ------------------------------------------------
            def q_proj(g):
                pq_t = pq_pool.tile([65, QT], f32, tag="pqo")
                pq = pq_t[0:64, :]
                for c in range(NCH):
                    nc.tensor.matmul(pq[:], wb[:, NCH * 128 + c * DK:
                                                NCH * 128 + (c + 1) * DK],
                                     xqs[:, g * TW + c * QT: g * TW + (c + 1) * QT],
                                     start=(c == 0), stop=(c == NCH - 1))
                nc.vector.tensor_copy(qTg[0:64, bass.ts(g, QT)], pq[:])
                nc.vector.tensor_copy(qTg[64:128, bass.ts(g, QT)],
                                      qTg[0:64, bass.ts(g, QT)])

            # k^T and v rows for one s-tile (phase 1: matmuls + copies)
            def kv_mm(t):
                pkv = pkv_pool.tile([128, QT], f32, tag="pkvt")
                for c in range(NCH):
                    nc.tensor.matmul(pkv[:], wb[:, bass.ts(c, 128)],
                                     xts[:, t * TW + c * QT: t * TW + (c + 1) * QT],
                                     start=(c == 0), stop=(c == NCH - 1))
                nc.vector.tensor_copy(kT[0:64, bass.ts(t, QT)], pkv[0:64, :])
                nc.vector.tensor_copy(kT[64:128, bass.ts(t, QT)],
                                      kT[0:64, bass.ts(t, QT)])
                vt = vts[:, (t % 2) * QT:(t % 2) * QT + QT]
                nc.vector.tensor_copy(vt[0:64, :], pkv[64:128, :])

            # phase 2 (emitted a couple of score-pairs later so the vts copy
            # latency hides behind score matmuls): v^T blocks into vaug
            def kv_tr(t):
                vt = vts[:, (t % 2) * QT:(t % 2) * QT + QT]
                # 66-element block stride keeps bf16 PSUM writes 4B-aligned
                pvt = pkv_pool.tile([128, 4 * 66], bf16, tag="pkvt")
                for blk in range(4):
                    nc.tensor.transpose(pvt[:, bass.ts(blk, 66)],
                                        vt[:, bass.ts(blk, 128)],
                                        ident[:])
                nc.vector.tensor_copy(vaug[:, t * 4 * 66:(t + 1) * 4 * 66], pvt[:])

            q_proj(0)
            kv_mm(0)
            load_mv(0)

            # flat 40-pair pipeline across all groups; masked pairs mid-group
            # (g3: early) so their deferred AVs drain inside later flushes.
            def group_order(g):
                unm = list(range(4 * g))
                msk = list(range(4 * g, 4 * g + 4))
                if not unm:
                    return msk
                if g == NG - 1:
                    return unm[:2] + msk + unm[2:]
                return unm[:-1] + msk + unm[-1:]

            flat = [(g, pi) for g in range(NG) for pi in group_order(g)]

            # projections are back-loaded into late (exp-bound) groups so the
            # PE has slack in early groups where exp is already saturated.
            sched = {
                0: [lambda: kv_tr(0)],
                1: [lambda: kv_mm(1)],
                2: [lambda: kv_tr(1)],
                3: [lambda: q_proj(1), lambda: load_mv(1)],
                5: [lambda: kv_mm(2)],
                6: [lambda: kv_tr(2)],
                7: [lambda: kv_mm(3)],
                9: [lambda: kv_tr(3)],
                10: [lambda: q_proj(2), lambda: load_mv(2)],
                14: [lambda: kv_mm(4)],
                16: [lambda: kv_tr(4), lambda: kv_mm(5)],
                18: [lambda: kv_tr(5)],
                20: [lambda: q_proj(3), lambda: load_mv(3)],
                22: [lambda: kv_mm(6)],
                24: [lambda: kv_tr(6), lambda: kv_mm(7)],
                26: [lambda: kv_tr(7)],
            }

            po = {}
            av_emitted = {g: 0 for g in range(NG)}
            deferred = []
            pending = None

            def finalize(g):
                # stage the unnormalized [65,512] group result and DMA it out
                ofin = fin_pool.tile([65, QT], bf16)
                nc.vector.tensor_copy(ofin[:], po[g][:])
                nc.sync.dma_start(out_d[g], ofin[:])

            def emit_av(g, pi, em):
                n_av = 2 * (4 * g + 4)      # total AV matmuls for this group
                for half in range(2):
                    sb = 2 * pi + half
                    nc.tensor.matmul(po[g][:], vaug[:, sb * 66:sb * 66 + 65],
                                     em[:, bass.ts(half, QT)],
                                     start=(av_emitted[g] == 0),
                                     stop=(av_emitted[g] == n_av - 1))
                    av_emitted[g] += 1
                if av_emitted[g] == n_av:
                    finalize(g)

            def flush():
                nonlocal pending
                if pending is None:
                    return
                # deferred masked AVs from earlier flushes are ready now
                for dg, dpi, dem in deferred:
                    emit_av(dg, dpi, dem)
                deferred.clear()
                ps, g, pi = pending
                pending = None
                em = exp_pool.tile([128, 2 * QT], bf16)
                nc.scalar.activation(em[:], ps[:], AF.Exp)
                if pi >= 4 * g:   # masked pair: multiplicative palette mask
                    for half in range(2):
                        rel = 2 * (pi - 4 * g) + half
                        off = mv[2 * g + rel // 4] - 128 * (rel % 4)
                        nc.vector.tensor_mul(
                            em[:, bass.ts(half, QT)],
                            em[:, bass.ts(half, QT)],
                            pal[:, bass.ds(off, QT)])
                    deferred.append((g, pi, em))
                else:
                    emit_av(g, pi, em)

            for idx, (g, pi) in enumerate(flat):
                if pi == group_order(g)[0]:
                    po[g] = pq_pool.tile([65, QT], f32, tag="pqo", name="po")
                ps = ps_pool.tile([128, 2 * QT], f32)
                for half in range(2):
                    sb = 2 * pi + half
                    rows = slice(64 * half, 64 * half + 64)
                    nc.tensor.matmul(ps[:, bass.ts(half, QT)],
                                     kT[rows, bass.ts(sb, 128)],
                                     qTg[rows, bass.ts(g, QT)],
                                     start=True, stop=True,
                                     tile_position=(64 * half, 0))
                flush()
                pending = (ps, g, pi)
                for thunk in sched.get(idx, []):
                    thunk()
            flush()
            for dg, dpi, dem in deferred:
                emit_av(dg, dpi, dem)
            deferred.clear()

        if kiter == 1:
            body()
        else:
            with tc.For_i(0, kiter, 1, staggered_reset=True):
                body()

    nc.compile()
    return nc


def _tile_cols(a):
    """[512, n*512] (d_e, cols) -> [n, 128, 4*512] tile-major host layout."""
    de, w = a.shape
    n = w // QT
    # out[t, p, c*QT + s] = a[c*128 + p, t*QT + s]
    return np.ascontiguousarray(
        a.reshape(NCH, 128, n, QT).transpose(2, 1, 0, 3).reshape(n, 128, NCH * QT))


def make_inputs(x, Wq, Wk, Wv):
    """Per-core input maps. x:[B,S,DE] f32; W*: [DE,64] f32."""
    wkv = np.concatenate([Wk, Wv], axis=1).astype(np.float32)          # [512,128]
    wqs = (Wq / np.float32(np.sqrt(DK))).astype(np.float32)            # [512,64]
    # weights chunk-major: [128, c*width + j] = W[c*128 + p, j]
    wkv_h = np.ascontiguousarray(
        wkv.reshape(NCH, 128, 128).transpose(1, 0, 2).reshape(128, NCH * 128))
    wq_h = np.ascontiguousarray(
        wqs.reshape(NCH, 128, DK).transpose(1, 0, 2).reshape(128, NCH * DK))
    wb_h = np.concatenate([wkv_h, wq_h], axis=1).astype(bfloat16)
    ident = np.eye(66, dtype=bfloat16)
    tri = (np.arange(896)[None, :] >= np.arange(128)[:, None] + 384).astype(bfloat16)
    in_maps = []
    for core in range(8):
        b, p = core // 2, core % 2
        xt = np.ascontiguousarray(x[b].T, dtype=np.float32)            # [512, 4096]
        cols = np.concatenate([np.arange(t * QT, (t + 1) * QT) for t in TQ[p]])
        moff = np.zeros((1, 8), dtype=np.int32)
        for g in range(NG):
            t = TQ[p][g]
            for jr in range(2):
                j = 2 * g + jr
                if j < t:
                    moff[0, 2 * g + jr] = PAL_KEEP
                elif j == t:
                    moff[0, 2 * g + jr] = PAL_TRI
                else:
                    moff[0, 2 * g + jr] = PAL_DROP
        in_maps.append(dict(xt=_tile_cols(xt).astype(bfloat16),
                            xq=_tile_cols(xt[:, cols]).astype(bfloat16),
                            wb=wb_h, moff=moff, tri=tri, ident=ident))
    return in_maps


def assemble(results):
    out = np.empty((B, S, DV), dtype=np.float32)
    for core in range(8):
        b, p = core // 2, core % 2
        o = results[core]["out"].astype(np.float32)   # [NG, 65, QT] bf16
        for g in range(NG):
            t = TQ[p][g]
            out[b, t * QT:(t + 1) * QT, :] = (o[g][0:64, :] / o[g][64:65, :]).T
    return out


_cache = {}


def _get_nc(kiter=1):
    if kiter not in _cache:
        _cache[kiter] = build(kiter)
    return _cache[kiter]


def run(x, Wq, Wk, Wv, kiter=1):
    nc = _get_nc(kiter)
    in_maps = make_inputs(x, Wq, Wk, Wv)
    res = run_bass_kernel_spmd(nc, in_maps, list(range(8)))
    return assemble(res.results)


def kernel(x, Wq, Wk, Wv):
    x = np.asarray(x, dtype=np.float32)
    return run(x, np.asarray(Wq, np.float32), np.asarray(Wk, np.float32),
               np.asarray(Wv, np.float32))
